# revision 30
# baseline (speedup 1.0000x reference)
"""LSTM cell (B=4096, D=U=2048) on 8 trn2 NeuronCores.

Tensor-parallel over units: core i computes units [i*256,(i+1)*256) of every
gate. Per core:
    z^T[1024 units, 4096 batch] = Wx_shard^T @ x^T + Wh_shard^T @ h^T
accumulated in PSUM (bf16 matmuls, fp32 accumulate), gate activations fused
with the bias add on ScalarE (units on partitions -> bias is per-partition),
elementwise LSTM combine on VectorE, outputs stored transposed and
re-transposed on the host.

v3: batch tiles 2..7 are processed in weight-stationary pairs — each
stationary weight tile feeds two consecutive matmuls (one per batch tile of
the pair), so walrus can skip every other LDWEIGHTS dispatch. Gates are
split into two waves per pair (g,i then f,o) of 4 PSUM banks each so wave
evacuation overlaps the other wave's matmuls, and each wave runs its x
k-phase before its h k-phase so the pair's x tiles die early enough to
prefetch the next pair within the same two SBUF buffers.
"""

import sys

sys.path.insert(0, "/opt/trn_rl_repo")

import ml_dtypes
import numpy as np

import concourse.bass as bass
import concourse.mybir as mybir
import concourse.tile as tile
from concourse.bass_utils import run_bass_kernel_spmd

B, D, U = 4096, 2048, 2048
N_CORES = 8
US = U // N_CORES          # units per core per gate (256)
UT = US // 128             # unit tiles of 128 per gate (2)
NB = 512                   # batch tile (free dim)
NT = B // NB               # batch tiles (8)
KX = D // 128              # k tiles for x gemm (16)
KH = U // 128              # k tiles for h gemm (16)
BF16 = mybir.dt.bfloat16
F32 = mybir.dt.float32
AF = mybir.ActivationFunctionType


def _split_excess_waits(nc, maxw=1):
    """This walrus build rejects instructions carrying more than one sem-wait
    ("Too many sync wait commands"), but Tile freely attaches several. Hoist
    the extra waits onto same-engine nops inserted right before the
    instruction — engine streams are in-order, so blocking semantics are
    identical."""
    cnt = 0
    for fn in nc.m.functions:
        for bb in fn.blocks:
            new_insts = []
            for inst in bb.instructions:
                si = inst.sync_info
                waits = list(si.on_wait) if si is not None else []
                if len(waits) > maxw:
                    for i in range(0, len(waits) - maxw, maxw):
                        nop = mybir.InstNoOp(name=f"syncsplit-{cnt}")
                        cnt += 1
                        nop.engine = inst.engine
                        nop.sync_info = mybir.SyncInfo(
                            on_wait=waits[i : i + maxw], on_update=[]
                        )
                        new_insts.append(nop)
                    si.on_wait = waits[len(waits) - maxw :]
                new_insts.append(inst)
            if len(new_insts) != len(bb.instructions):
                bb.instructions = new_insts
    return cnt


def build_nc() -> bass.Bass:
    nc = bass.Bass()

    xT = nc.dram_tensor("xT", [D, B], BF16, kind="ExternalInput")
    hT = nc.dram_tensor("hT", [U, B], BF16, kind="ExternalInput")
    wx = nc.dram_tensor("wx", [D, 4 * US], BF16, kind="ExternalInput")
    wh = nc.dram_tensor("wh", [U, 4 * US], BF16, kind="ExternalInput")
    # bias, host-prepped to [128, 8]: column j = units [j*128,(j+1)*128) of
    # the concatenated [f,i,o,g] 1024-unit block (gate j//2, unit-tile j%2)
    bias = nc.dram_tensor("bias", [128, 4 * UT], F32, kind="ExternalInput")
    # c input and both outputs travel as bf16: halves DMA traffic and the
    # final output drain; costs ~1e-3 rel err (budget is 2e-2)
    cT = nc.dram_tensor("cT", [US, B], BF16, kind="ExternalInput")
    h_newT = nc.dram_tensor("h_newT", [US, B], BF16, kind="ExternalOutput")
    c_newT = nc.dram_tensor("c_newT", [US, B], BF16, kind="ExternalOutput")

    wx_r = wx.rearrange("(kt p) u -> p kt u", p=128)  # [128, KX, 1024]
    wh_r = wh.rearrange("(kt p) u -> p kt u", p=128)
    xT_r = xT.rearrange("(kt p) b -> p kt b", p=128)  # [128, KX, B]
    hT_r = hT.rearrange("(kt p) b -> p kt b", p=128)

    with tile.TileContext(nc) as tc:
        with (
            tc.tile_pool(name="wpool", bufs=1) as wpool,
            tc.tile_pool(name="singles", bufs=1) as singles,
            tc.tile_pool(name="acts", bufs=2) as apool,
            tc.tile_pool(name="ew", bufs=3) as epool,
            tc.tile_pool(name="psum", bufs=8, space="PSUM") as ppool,
        ):
            # Startup on a single HWDGE ring (FIFO): first x chunk, then Wx
            # k-tiles interleaved with the remaining x chunks, so the first
            # batch tile's k-outer matmuls track the arrival stream. The
            # first-tile x/h live in per-chunk tiles (4 k-tiles each) for
            # fine-grained deps; later tiles use whole tiles.
            CH = 4  # k-tiles per startup chunk
            chunks = [(0, 1), (1, 2), (2, 4)] + [(j * CH, (j + 1) * CH) for j in range(1, KX // CH)]
            x0c = {}
            h0c = {}
            wx_t = []
            wh_t = []
            wx0_parts = {}  # gi -> weight AP for kt=0 (split first tile)
            nsl0 = bass.ts(0, NB)
            for (k0, k1) in chunks:
                xc = apool.tile(
                    [128, k1 - k0, NB], BF16, tag=f"x0c{k0}", bufs=1, name=f"x0c{k0}"
                )
                nc.sync.dma_start(out=xc[:], in_=xT_r[:, k0:k1, nsl0])
                for kt in range(k0, k1):
                    x0c[kt] = xc[:, kt - k0, :]
                for kt in range(k0, k1):
                    if kt < 2:
                        # split the first weight k-tiles: each 64KB g-gate
                        # piece lands first so the k-outer matmuls track the
                        # arrival stream with finer granularity
                        whi = wpool.tile([128, US], BF16, tag=f"wx{kt}hi")
                        nc.sync.dma_start(out=whi[:], in_=wx_r[:, kt, 3 * US :])
                        wlo = wpool.tile([128, 3 * US], BF16, tag=f"wx{kt}lo")
                        nc.sync.dma_start(out=wlo[:], in_=wx_r[:, kt, : 3 * US])
                        wx0_parts[(kt, 3)] = whi
                        for gi in range(3):
                            wx0_parts[(kt, gi)] = wlo[:, gi * US : (gi + 1) * US]
                        wx_t.append(None)
                    else:
                        wt = wpool.tile([128, 4 * US], BF16, tag=f"wx{kt}")
                        nc.sync.dma_start(out=wt[:], in_=wx_r[:, kt, :])
                        wx_t.append(wt)
            b_sb = singles.tile([128, 4 * UT], F32)
            nc.sync.dma_start(out=b_sb[:], in_=bias[:])
            for j in range(KH // CH):
                hc = apool.tile(
                    [128, CH, NB], BF16, tag=f"h0c{j}", bufs=1, name=f"h0c{j}"
                )
                nc.sync.dma_start(
                    out=hc[:], in_=hT_r[:, j * CH : (j + 1) * CH, nsl0]
                )
                for kt in range(j * CH, (j + 1) * CH):
                    h0c[kt] = hc[:, kt - j * CH, :]
                for kt in range(j * CH, (j + 1) * CH):
                    wt = wpool.tile([128, 4 * US], BF16, tag=f"wh{kt}")
                    nc.sync.dma_start(out=wt[:], in_=wh_r[:, kt, :])
                    wh_t.append(wt)

            # MM groups run in order [g, i, f, o]; each gate is consumed as
            # soon as possible so only o's short chain trails the last matmul
            GATE_ORDER = (3, 1, 0, 2)  # gi of g, i, f, o in weight layout
            WAVE_A = (3, 1)            # g, i
            WAVE_B = (0, 2)            # f, o

            def wx_ap(kt, gi, ut):
                if kt < 2:
                    return wx0_parts[(kt, gi)][:, ut * 128 : (ut + 1) * 128]
                c0 = gi * US + ut * 128
                return wx_t[kt][:, c0 : c0 + 128]

            def act_gate(ps, gi, ut, name, w=NB):
                # o-gate tiles are bf16 (they feed the bf16 h_new output);
                # f/i/g stay fp32 for the c_new accumulate path
                dt = BF16 if gi == 2 else F32
                g_sb = epool.tile(
                    [128, w], dt, tag=f"gate{gi}_{w}", name=name
                )
                nc.scalar.activation(
                    g_sb[:],
                    ps[:],
                    AF.Tanh if gi == 3 else AF.Sigmoid,
                    bias=b_sb[:, gi * UT + ut : gi * UT + ut + 1],
                )
                return g_sb

            def elementwise(pss, n, ut):
                # pss indexed by weight-layout gi; groups complete in
                # GATE_ORDER, so evaluate the LSTM chain in that order
                nsl = bass.ts(n, NB)
                usl = slice(ut * 128, (ut + 1) * 128)
                c_sb = epool.tile([128, NB], BF16, tag="c_sb", name="c_sb")
                nc.sync.dma_start(out=c_sb[:], in_=cT[usl, nsl])
                g_t = act_gate(pss[3], 3, ut, "g_t")
                i_t = act_gate(pss[1], 1, ut, "i_t")
                nc.vector.tensor_mul(i_t[:], i_t[:], g_t[:])      # i*g
                f_t = act_gate(pss[0], 0, ut, "f_t")
                nc.vector.tensor_mul(f_t[:], f_t[:], c_sb[:])     # f*c
                cn = epool.tile([128, NB], BF16, tag="cn", name="cn")
                nc.vector.tensor_add(cn[:], f_t[:], i_t[:])       # c_new
                nc.sync.dma_start(out=c_newT[usl, nsl], in_=cn[:])
                tn = epool.tile([128, NB], BF16, tag="tnb", name="tn")
                nc.scalar.activation(tn[:], cn[:], AF.Tanh)       # tanh(c_new)
                o_t = act_gate(pss[2], 2, ut, "o_t")
                nc.vector.tensor_mul(o_t[:], o_t[:], tn[:])       # h_new
                nc.sync.dma_start(out=h_newT[usl, nsl], in_=o_t[:])

            def stage1(psA, ti, ut, tname):
                # after wave A (g,i) stops: compute ig = sigmoid(i)*tanh(g),
                # freeing wave A's PSUM banks while wave B still matmuls
                g_t = epool.tile([128, NB], F32, tag="gate3", name=f"g_{tname}")
                nc.scalar.activation(
                    g_t[:], psA[3][ti][:], AF.Tanh,
                    bias=b_sb[:, 3 * UT + ut : 3 * UT + ut + 1],
                )
                ig = epool.tile([128, NB], F32, tag="ig", name=f"ig_{tname}")
                nc.scalar.activation(
                    ig[:], psA[1][ti][:], AF.Sigmoid,
                    bias=b_sb[:, 1 * UT + ut : 1 * UT + ut + 1],
                )
                nc.vector.tensor_mul(ig[:], ig[:], g_t[:])
                return ig

            def stage2(psB, ti, ig, n, ut):
                # after wave B (f,o) stops: finish the LSTM combine
                nsl = bass.ts(n, NB)
                usl = slice(ut * 128, (ut + 1) * 128)
                c_sb = epool.tile([128, NB], BF16, tag="c_sb", name="c_sb")
                nc.sync.dma_start(out=c_sb[:], in_=cT[usl, nsl])
                f_t = act_gate(psB[0][ti], 0, ut, "f_t")
                nc.vector.tensor_mul(f_t[:], f_t[:], c_sb[:])     # f*c
                cn = epool.tile([128, NB], BF16, tag="cn", name="cn")
                nc.vector.tensor_add(cn[:], f_t[:], ig[:])        # c_new
                nc.sync.dma_start(out=c_newT[usl, nsl], in_=cn[:])
                tn = epool.tile([128, NB], BF16, tag="tnb", name="tn")
                nc.scalar.activation(tn[:], cn[:], AF.Tanh)       # tanh(c_new)
                o_t = act_gate(psB[2][ti], 2, ut, "o_t")
                nc.vector.tensor_mul(o_t[:], o_t[:], tn[:])       # h_new
                nc.sync.dma_start(out=h_newT[usl, nsl], in_=o_t[:])

            # --- n = 0: k-outer over all 8 (ut, gate) groups, one PSUM bank
            # each, so every arriving weight k-tile feeds 8 matmuls and the
            # PE tracks the weight-load stream instead of stalling on it.
            ps_all = [
                [
                    ppool.tile([128, NB], F32, tag="ps", name=f"ps{ut}{gi}")
                    for gi in range(4)
                ]
                for ut in range(UT)
            ]
            for kt in range(KX):
                if kt < 2:
                    # gate g first across both ut groups: its 64KB weight
                    # piece is the first to land
                    order = [(ut, 3) for ut in range(UT)] + [
                        (ut, gi) for gi in (1, 0, 2) for ut in range(UT)
                    ]
                else:
                    order = [(ut, gi) for ut in range(UT) for gi in GATE_ORDER]
                for ut, gi in order:
                    nc.tensor.matmul(
                        ps_all[ut][gi][:],
                        wx_ap(kt, gi, ut),
                        x0c[kt],
                        start=(kt == 0),
                        stop=False,
                    )
            for kt in range(KH):
                for ut in range(UT):
                    for gi in GATE_ORDER:
                        c0 = gi * US + ut * 128
                        nc.tensor.matmul(
                            ps_all[ut][gi][:],
                            wh_t[kt][:, c0 : c0 + 128],
                            h0c[kt],
                            start=False,
                            stop=(kt == KH - 1),
                        )
            for ut in range(UT):
                elementwise(ps_all[ut], 0, ut)

            # --- n = 1: gate-outer, k-inner; x from per-chunk tiles (reusing
            # the n=0 chunk buffers, which are dead by now), h from a whole
            # tile. Keeps the h_sb/x_sb whole-tile buffers free for the pairs.
            x1c = {}
            for (k0, k1) in chunks:
                xc = apool.tile(
                    [128, k1 - k0, NB], BF16, tag=f"x0c{k0}", bufs=1, name=f"x1c{k0}"
                )
                nc.sync.dma_start(out=xc[:], in_=xT_r[:, k0:k1, bass.ts(1, NB)])
                for kt in range(k0, k1):
                    x1c[kt] = xc[:, kt - k0, :]
            h1_sb = apool.tile([128, KH, NB], BF16, tag="h_sb", name="h1_sb")
            nc.sync.dma_start(out=h1_sb[:], in_=hT_r[:, :, bass.ts(1, NB)])
            for ut in range(UT):
                pss = [
                    ppool.tile([128, NB], F32, tag="ps", name=f"ps{gi}")
                    for gi in range(4)
                ]
                for gi in GATE_ORDER:
                    c0 = gi * US + ut * 128
                    for kt in range(KX):
                        nc.tensor.matmul(
                            pss[gi][:],
                            wx_ap(kt, gi, ut),
                            x1c[kt],
                            start=(kt == 0),
                            stop=False,
                        )
                    for kt in range(KH):
                        nc.tensor.matmul(
                            pss[gi][:],
                            wh_t[kt][:, c0 : c0 + 128],
                            h1_sb[:, kt, :],
                            start=False,
                            stop=(kt == KH - 1),
                        )
                elementwise(pss, 1, ut)

            # --- n = 2..7 in weight-stationary pairs: per (ut, wave, k) one
            # stationary weight slice feeds both batch tiles' matmuls
            # back-to-back (walrus skips the second LDWEIGHTS). Each wave
            # runs its x phase before its h phase; x tiles die after ut=1's
            # wave-B x phase so the next pair's x prefetch fits in the same
            # two buffers.
            for (na, nb) in ((2, 3), (4, 5), (6, 7)):
                last_pair = nb == NT - 1
                xa = apool.tile([128, KX, NB], BF16, tag="x_sb", name=f"x{na}")
                nc.sync.dma_start(out=xa[:], in_=xT_r[:, :, bass.ts(na, NB)])
                xb = apool.tile([128, KX, NB], BF16, tag="x_sb", name=f"x{nb}")
                nc.sync.dma_start(out=xb[:], in_=xT_r[:, :, bass.ts(nb, NB)])
                ha = apool.tile([128, KH, NB], BF16, tag="h_sb", name=f"h{na}")
                nc.sync.dma_start(out=ha[:], in_=hT_r[:, :, bass.ts(na, NB)])
                hb = apool.tile([128, KH, NB], BF16, tag="h_sb", name=f"h{nb}")
                nc.sync.dma_start(out=hb[:], in_=hT_r[:, :, bass.ts(nb, NB)])

                for ut in range(UT):
                    if last_pair and ut == UT - 1:
                        # kernel finale: per-tile gate-outer (no weight
                        # sharing); tile b's o-gate is split 384/128 so only
                        # a 128-wide act+mul+store trails the last matmul
                        usl = slice(ut * 128, (ut + 1) * 128)
                        pss = [
                            ppool.tile([128, NB], F32, tag="ps", name=f"ps{gi}")
                            for gi in range(4)
                        ]
                        for gi in GATE_ORDER:
                            c0 = gi * US + ut * 128
                            for kt in range(KX):
                                nc.tensor.matmul(
                                    pss[gi][:], wx_ap(kt, gi, ut), xa[:, kt, :],
                                    start=(kt == 0), stop=False,
                                )
                            for kt in range(KH):
                                nc.tensor.matmul(
                                    pss[gi][:], wh_t[kt][:, c0 : c0 + 128],
                                    ha[:, kt, :],
                                    start=False, stop=(kt == KH - 1),
                                )
                        elementwise(pss, na, ut)

                        nslb = bass.ts(nb, NB)
                        psb = [
                            ppool.tile([128, NB], F32, tag="ps", name=f"psb{gi}")
                            for gi in range(4)
                        ]
                        for gi in (3, 1, 0):
                            c0 = gi * US + ut * 128
                            for kt in range(KX):
                                nc.tensor.matmul(
                                    psb[gi][:], wx_ap(kt, gi, ut), xb[:, kt, :],
                                    start=(kt == 0), stop=False,
                                )
                            for kt in range(KH):
                                nc.tensor.matmul(
                                    psb[gi][:], wh_t[kt][:, c0 : c0 + 128],
                                    hb[:, kt, :],
                                    start=False, stop=(kt == KH - 1),
                                )
                        # combine chain for c_new runs during the o loops
                        c_sb = epool.tile([128, NB], BF16, tag="c_sb", name="c_sb")
                        nc.sync.dma_start(out=c_sb[:], in_=cT[usl, nslb])
                        g_t = act_gate(psb[3], 3, ut, "g_t")
                        i_t = act_gate(psb[1], 1, ut, "i_t")
                        nc.vector.tensor_mul(i_t[:], i_t[:], g_t[:])
                        f_t = act_gate(psb[0], 0, ut, "f_t")
                        nc.vector.tensor_mul(f_t[:], f_t[:], c_sb[:])
                        cn = epool.tile([128, NB], BF16, tag="cn", name="cn")
                        nc.vector.tensor_add(cn[:], f_t[:], i_t[:])
                        nc.sync.dma_start(out=c_newT[usl, nslb], in_=cn[:])
                        tn = epool.tile([128, NB], BF16, tag="tnb", name="tn")
                        nc.scalar.activation(tn[:], cn[:], AF.Tanh)
                        # o gate, wide part: evacuates while the narrow part
                        # is still matmuling
                        ps_o2 = ppool.tile([128, NB], F32, tag="ps", name="ps_o2")
                        c0o = 2 * US + ut * 128
                        for kt in range(KX):
                            nc.tensor.matmul(
                                psb[2][:, 0:384], wx_ap(kt, 2, ut),
                                xb[:, kt, 0:384],
                                start=(kt == 0), stop=False,
                            )
                        for kt in range(KH):
                            nc.tensor.matmul(
                                psb[2][:, 0:384], wh_t[kt][:, c0o : c0o + 128],
                                hb[:, kt, 0:384],
                                start=False, stop=(kt == KH - 1),
                            )
                        o1 = act_gate(psb[2][:, 0:384], 2, ut, "o1", w=384)
                        nc.vector.tensor_mul(o1[:], o1[:], tn[:, 0:384])
                        nc.sync.dma_start(
                            out=h_newT[usl, nb * NB : nb * NB + 384], in_=o1[:]
                        )
                        for kt in range(KX):
                            nc.tensor.matmul(
                                ps_o2[:, 0:128], wx_ap(kt, 2, ut),
                                xb[:, kt, 384:512],
                                start=(kt == 0), stop=False,
                            )
                        for kt in range(KH):
                            nc.tensor.matmul(
                                ps_o2[:, 0:128], wh_t[kt][:, c0o : c0o + 128],
                                hb[:, kt, 384:512],
                                start=False, stop=(kt == KH - 1),
                            )
                        o2 = act_gate(ps_o2[:, 0:128], 2, ut, "o2", w=128)
                        nc.vector.tensor_mul(o2[:], o2[:], tn[:, 384:512])
                        nc.sync.dma_start(
                            out=h_newT[usl, nb * NB + 384 : (nb + 1) * NB],
                            in_=o2[:],
                        )
                        continue
                    psA = {
                        gi: [
                            ppool.tile([128, NB], F32, tag="ps", name=f"ps{gi}{t}")
                            for t in "ab"
                        ]
                        for gi in WAVE_A
                    }
                    for kt in range(KX):
                        for gi in WAVE_A:
                            w = wx_ap(kt, gi, ut)
                            nc.tensor.matmul(
                                psA[gi][0][:], w, xa[:, kt, :],
                                start=(kt == 0), stop=False,
                            )
                            nc.tensor.matmul(
                                psA[gi][1][:], w, xb[:, kt, :],
                                start=(kt == 0), stop=False,
                            )
                    psB = {
                        gi: [
                            ppool.tile([128, NB], F32, tag="ps", name=f"ps{gi}{t}")
                            for t in "ab"
                        ]
                        for gi in WAVE_B
                    }
                    for kt in range(KX):
                        for gi in WAVE_B:
                            w = wx_ap(kt, gi, ut)
                            nc.tensor.matmul(
                                psB[gi][0][:], w, xa[:, kt, :],
                                start=(kt == 0), stop=False,
                            )
                            nc.tensor.matmul(
                                psB[gi][1][:], w, xb[:, kt, :],
                                start=(kt == 0), stop=False,
                            )
                    for kt in range(KH):
                        for gi in WAVE_A:
                            c0 = gi * US + ut * 128
                            w = wh_t[kt][:, c0 : c0 + 128]
                            nc.tensor.matmul(
                                psA[gi][0][:], w, ha[:, kt, :],
                                start=False, stop=(kt == KH - 1),
                            )
                            nc.tensor.matmul(
                                psA[gi][1][:], w, hb[:, kt, :],
                                start=False, stop=(kt == KH - 1),
                            )
                    ig_a = stage1(psA, 0, ut, f"a{ut}")
                    ig_b = stage1(psA, 1, ut, f"b{ut}")
                    for kt in range(KH):
                        for gi in WAVE_B:
                            c0 = gi * US + ut * 128
                            w = wh_t[kt][:, c0 : c0 + 128]
                            nc.tensor.matmul(
                                psB[gi][0][:], w, ha[:, kt, :],
                                start=False, stop=(kt == KH - 1),
                            )
                            nc.tensor.matmul(
                                psB[gi][1][:], w, hb[:, kt, :],
                                start=False, stop=(kt == KH - 1),
                            )
                    stage2(psB, 0, ig_a, na, ut)
                    stage2(psB, 1, ig_b, nb, ut)
    _split_excess_waits(nc)
    return nc


_NC_CACHE = None


def _get_nc():
    global _NC_CACHE
    if _NC_CACHE is None:
        _NC_CACHE = build_nc()
    return _NC_CACHE


def make_in_maps(x, h, c, Wxf, Wxi, Wxo, Wxg, bf, bi, bo, bg, Whf, Whi, Who, Whg):
    bf16 = ml_dtypes.bfloat16
    xT = np.ascontiguousarray(np.asarray(x, np.float32).T).astype(bf16)
    hT = np.ascontiguousarray(np.asarray(h, np.float32).T).astype(bf16)
    c = np.asarray(c, np.float32)
    Wx = np.stack([np.asarray(w, np.float32) for w in (Wxf, Wxi, Wxo, Wxg)])
    Wh = np.stack([np.asarray(w, np.float32) for w in (Whf, Whi, Who, Whg)])
    bias = np.stack([np.asarray(v, np.float32) for v in (bf, bi, bo, bg)])

    in_maps = []
    for i in range(N_CORES):
        s = slice(i * US, (i + 1) * US)
        wx_i = np.concatenate([Wx[g, :, s] for g in range(4)], axis=1).astype(bf16)
        wh_i = np.concatenate([Wh[g, :, s] for g in range(4)], axis=1).astype(bf16)
        b_i = np.concatenate([bias[g, s] for g in range(4)])  # [1024]
        b_i = np.ascontiguousarray(b_i.reshape(4 * UT, 128).T)  # [128, 8]
        cT_i = np.ascontiguousarray(c[:, s].T).astype(bf16)  # [US, B]
        in_maps.append(
            {"xT": xT, "hT": hT, "wx": wx_i, "wh": wh_i, "bias": b_i, "cT": cT_i}
        )
    return in_maps


def run(in_maps, **kwargs):
    nc = _get_nc()
    return run_bass_kernel_spmd(nc, in_maps, list(range(N_CORES)), **kwargs)


def gather(results):
    h_new = np.empty((B, U), np.float32)
    c_new = np.empty((B, U), np.float32)
    for i in range(N_CORES):
        s = slice(i * US, (i + 1) * US)
        h_new[:, s] = results[i]["h_newT"].astype(np.float32).T
        c_new[:, s] = results[i]["c_newT"].astype(np.float32).T
    return h_new, c_new


def kernel(**inputs):
    res = run(make_in_maps(**inputs))
    return gather(res.results)


# revision 31
# speedup vs baseline: 1.0016x; 1.0016x over previous
"""LSTM cell (B=4096, D=U=2048) on 8 trn2 NeuronCores.

Tensor-parallel over units: core i computes units [i*256,(i+1)*256) of every
gate. Per core:
    z^T[1024 units, 4096 batch] = Wx_shard^T @ x^T + Wh_shard^T @ h^T
accumulated in PSUM (bf16 matmuls, fp32 accumulate), gate activations fused
with the bias add on ScalarE (units on partitions -> bias is per-partition),
elementwise LSTM combine on VectorE, outputs stored transposed and
re-transposed on the host.

v3: batch tiles 2..7 are processed in weight-stationary pairs — each
stationary weight tile feeds two consecutive matmuls (one per batch tile of
the pair), so walrus can skip every other LDWEIGHTS dispatch. Gates are
split into two waves per pair (g,i then f,o) of 4 PSUM banks each so wave
evacuation overlaps the other wave's matmuls, and each wave runs its x
k-phase before its h k-phase so the pair's x tiles die early enough to
prefetch the next pair within the same two SBUF buffers.
"""

import sys

sys.path.insert(0, "/opt/trn_rl_repo")

import ml_dtypes
import numpy as np

import concourse.bass as bass
import concourse.mybir as mybir
import concourse.tile as tile
from concourse.bass_utils import run_bass_kernel_spmd

B, D, U = 4096, 2048, 2048
N_CORES = 8
US = U // N_CORES          # units per core per gate (256)
UT = US // 128             # unit tiles of 128 per gate (2)
NB = 512                   # batch tile (free dim)
NT = B // NB               # batch tiles (8)
KX = D // 128              # k tiles for x gemm (16)
KH = U // 128              # k tiles for h gemm (16)
BF16 = mybir.dt.bfloat16
F32 = mybir.dt.float32
AF = mybir.ActivationFunctionType


def _split_excess_waits(nc, maxw=1):
    """This walrus build rejects instructions carrying more than one sem-wait
    ("Too many sync wait commands"), but Tile freely attaches several. Hoist
    the extra waits onto same-engine nops inserted right before the
    instruction — engine streams are in-order, so blocking semantics are
    identical."""
    cnt = 0
    for fn in nc.m.functions:
        for bb in fn.blocks:
            new_insts = []
            for inst in bb.instructions:
                si = inst.sync_info
                waits = list(si.on_wait) if si is not None else []
                if len(waits) > maxw:
                    for i in range(0, len(waits) - maxw, maxw):
                        nop = mybir.InstNoOp(name=f"syncsplit-{cnt}")
                        cnt += 1
                        nop.engine = inst.engine
                        nop.sync_info = mybir.SyncInfo(
                            on_wait=waits[i : i + maxw], on_update=[]
                        )
                        new_insts.append(nop)
                    si.on_wait = waits[len(waits) - maxw :]
                new_insts.append(inst)
            if len(new_insts) != len(bb.instructions):
                bb.instructions = new_insts
    return cnt


def build_nc() -> bass.Bass:
    nc = bass.Bass()

    xT = nc.dram_tensor("xT", [D, B], BF16, kind="ExternalInput")
    hT = nc.dram_tensor("hT", [U, B], BF16, kind="ExternalInput")
    wx = nc.dram_tensor("wx", [D, 4 * US], BF16, kind="ExternalInput")
    wh = nc.dram_tensor("wh", [U, 4 * US], BF16, kind="ExternalInput")
    # bias, host-prepped to [128, 8]: column j = units [j*128,(j+1)*128) of
    # the concatenated [f,i,o,g] 1024-unit block (gate j//2, unit-tile j%2)
    bias = nc.dram_tensor("bias", [128, 4 * UT], F32, kind="ExternalInput")
    # c input and both outputs travel as bf16: halves DMA traffic and the
    # final output drain; costs ~1e-3 rel err (budget is 2e-2)
    cT = nc.dram_tensor("cT", [US, B], BF16, kind="ExternalInput")
    h_newT = nc.dram_tensor("h_newT", [US, B], BF16, kind="ExternalOutput")
    c_newT = nc.dram_tensor("c_newT", [US, B], BF16, kind="ExternalOutput")

    wx_r = wx.rearrange("(kt p) u -> p kt u", p=128)  # [128, KX, 1024]
    wh_r = wh.rearrange("(kt p) u -> p kt u", p=128)
    xT_r = xT.rearrange("(kt p) b -> p kt b", p=128)  # [128, KX, B]
    hT_r = hT.rearrange("(kt p) b -> p kt b", p=128)

    with tile.TileContext(nc) as tc:
        with (
            tc.tile_pool(name="wpool", bufs=1) as wpool,
            tc.tile_pool(name="singles", bufs=1) as singles,
            tc.tile_pool(name="acts", bufs=2) as apool,
            tc.tile_pool(name="ew", bufs=3) as epool,
            tc.tile_pool(name="psum", bufs=8, space="PSUM") as ppool,
        ):
            # Startup on a single HWDGE ring (FIFO): first x chunk, then Wx
            # k-tiles interleaved with the remaining x chunks, so the first
            # batch tile's k-outer matmuls track the arrival stream. The
            # first-tile x/h live in per-chunk tiles (4 k-tiles each) for
            # fine-grained deps; later tiles use whole tiles.
            CH = 4  # k-tiles per startup chunk
            chunks = [(0, 1), (1, 2), (2, 4)] + [(j * CH, (j + 1) * CH) for j in range(1, KX // CH)]
            x0c = {}
            h0c = {}
            wx_t = []
            wh_t = []
            wx0_parts = {}  # gi -> weight AP for kt=0 (split first tile)
            nsl0 = bass.ts(0, NB)
            for (k0, k1) in chunks:
                xc = apool.tile(
                    [128, k1 - k0, NB], BF16, tag=f"x0c{k0}", bufs=1, name=f"x0c{k0}"
                )
                nc.sync.dma_start(out=xc[:], in_=xT_r[:, k0:k1, nsl0])
                for kt in range(k0, k1):
                    x0c[kt] = xc[:, kt - k0, :]
                for kt in range(k0, k1):
                    if kt < 2:
                        # split the first weight k-tiles: each 64KB g-gate
                        # piece lands first so the k-outer matmuls track the
                        # arrival stream with finer granularity
                        whi = wpool.tile([128, US], BF16, tag=f"wx{kt}hi")
                        nc.sync.dma_start(out=whi[:], in_=wx_r[:, kt, 3 * US :])
                        wlo = wpool.tile([128, 3 * US], BF16, tag=f"wx{kt}lo")
                        nc.sync.dma_start(out=wlo[:], in_=wx_r[:, kt, : 3 * US])
                        wx0_parts[(kt, 3)] = whi
                        for gi in range(3):
                            wx0_parts[(kt, gi)] = wlo[:, gi * US : (gi + 1) * US]
                        wx_t.append(None)
                    else:
                        wt = wpool.tile([128, 4 * US], BF16, tag=f"wx{kt}")
                        nc.sync.dma_start(out=wt[:], in_=wx_r[:, kt, :])
                        wx_t.append(wt)
            b_sb = singles.tile([128, 4 * UT], F32)
            nc.sync.dma_start(out=b_sb[:], in_=bias[:])
            for j in range(KH // CH):
                hc = apool.tile(
                    [128, CH, NB], BF16, tag=f"h0c{j}", bufs=1, name=f"h0c{j}"
                )
                nc.sync.dma_start(
                    out=hc[:], in_=hT_r[:, j * CH : (j + 1) * CH, nsl0]
                )
                for kt in range(j * CH, (j + 1) * CH):
                    h0c[kt] = hc[:, kt - j * CH, :]
                for kt in range(j * CH, (j + 1) * CH):
                    wt = wpool.tile([128, 4 * US], BF16, tag=f"wh{kt}")
                    nc.sync.dma_start(out=wt[:], in_=wh_r[:, kt, :])
                    wh_t.append(wt)

            # MM groups run in order [g, i, f, o]; each gate is consumed as
            # soon as possible so only o's short chain trails the last matmul
            GATE_ORDER = (3, 1, 0, 2)  # gi of g, i, f, o in weight layout
            WAVE_A = (3, 1)            # g, i
            WAVE_B = (0, 2)            # f, o

            def wx_ap(kt, gi, ut):
                if kt < 2:
                    return wx0_parts[(kt, gi)][:, ut * 128 : (ut + 1) * 128]
                c0 = gi * US + ut * 128
                return wx_t[kt][:, c0 : c0 + 128]

            def act_gate(ps, gi, ut, name, w=NB):
                # o-gate tiles are bf16 (they feed the bf16 h_new output);
                # f/i/g stay fp32 for the c_new accumulate path
                dt = BF16 if gi == 2 else F32
                g_sb = epool.tile(
                    [128, w], dt, tag=f"gate{gi}_{w}", name=name
                )
                nc.scalar.activation(
                    g_sb[:],
                    ps[:],
                    AF.Tanh if gi == 3 else AF.Sigmoid,
                    bias=b_sb[:, gi * UT + ut : gi * UT + ut + 1],
                )
                return g_sb

            def elementwise(pss, n, ut):
                # pss indexed by weight-layout gi; groups complete in
                # GATE_ORDER, so evaluate the LSTM chain in that order
                nsl = bass.ts(n, NB)
                usl = slice(ut * 128, (ut + 1) * 128)
                c_sb = epool.tile([128, NB], BF16, tag="c_sb", name="c_sb")
                nc.sync.dma_start(out=c_sb[:], in_=cT[usl, nsl])
                g_t = act_gate(pss[3], 3, ut, "g_t")
                i_t = act_gate(pss[1], 1, ut, "i_t")
                nc.vector.tensor_mul(i_t[:], i_t[:], g_t[:])      # i*g
                f_t = act_gate(pss[0], 0, ut, "f_t")
                nc.vector.tensor_mul(f_t[:], f_t[:], c_sb[:])     # f*c
                cn = epool.tile([128, NB], BF16, tag="cn", name="cn")
                nc.vector.tensor_add(cn[:], f_t[:], i_t[:])       # c_new
                nc.sync.dma_start(out=c_newT[usl, nsl], in_=cn[:])
                tn = epool.tile([128, NB], BF16, tag="tnb", name="tn")
                nc.scalar.activation(tn[:], cn[:], AF.Tanh)       # tanh(c_new)
                o_t = act_gate(pss[2], 2, ut, "o_t")
                nc.vector.tensor_mul(o_t[:], o_t[:], tn[:])       # h_new
                nc.sync.dma_start(out=h_newT[usl, nsl], in_=o_t[:])

            def stage1(psA, ti, ut, tname):
                # after wave A (g,i) stops: compute ig = sigmoid(i)*tanh(g),
                # freeing wave A's PSUM banks while wave B still matmuls
                g_t = epool.tile([128, NB], F32, tag="gate3", name=f"g_{tname}")
                nc.scalar.activation(
                    g_t[:], psA[3][ti][:], AF.Tanh,
                    bias=b_sb[:, 3 * UT + ut : 3 * UT + ut + 1],
                )
                ig = epool.tile([128, NB], F32, tag="ig", name=f"ig_{tname}")
                nc.scalar.activation(
                    ig[:], psA[1][ti][:], AF.Sigmoid,
                    bias=b_sb[:, 1 * UT + ut : 1 * UT + ut + 1],
                )
                nc.vector.tensor_mul(ig[:], ig[:], g_t[:])
                return ig

            def stage2(psB, ti, ig, n, ut):
                # after wave B (f,o) stops: finish the LSTM combine
                nsl = bass.ts(n, NB)
                usl = slice(ut * 128, (ut + 1) * 128)
                c_sb = epool.tile([128, NB], BF16, tag="c_sb", name="c_sb")
                nc.sync.dma_start(out=c_sb[:], in_=cT[usl, nsl])
                f_t = act_gate(psB[0][ti], 0, ut, "f_t")
                nc.vector.tensor_mul(f_t[:], f_t[:], c_sb[:])     # f*c
                cn = epool.tile([128, NB], BF16, tag="cn", name="cn")
                nc.vector.tensor_add(cn[:], f_t[:], ig[:])        # c_new
                nc.sync.dma_start(out=c_newT[usl, nsl], in_=cn[:])
                tn = epool.tile([128, NB], BF16, tag="tnb", name="tn")
                nc.scalar.activation(tn[:], cn[:], AF.Tanh)       # tanh(c_new)
                o_t = act_gate(psB[2][ti], 2, ut, "o_t")
                nc.vector.tensor_mul(o_t[:], o_t[:], tn[:])       # h_new
                nc.sync.dma_start(out=h_newT[usl, nsl], in_=o_t[:])

            # --- n = 0: k-outer over all 8 (ut, gate) groups, one PSUM bank
            # each, so every arriving weight k-tile feeds 8 matmuls and the
            # PE tracks the weight-load stream instead of stalling on it.
            ps_all = [
                [
                    ppool.tile([128, NB], F32, tag="ps", name=f"ps{ut}{gi}")
                    for gi in range(4)
                ]
                for ut in range(UT)
            ]
            for kt in range(KX):
                if kt < 2:
                    # gate g first across both ut groups: its 64KB weight
                    # piece is the first to land
                    order = [(ut, 3) for ut in range(UT)] + [
                        (ut, gi) for gi in (1, 0, 2) for ut in range(UT)
                    ]
                else:
                    order = [(ut, gi) for ut in range(UT) for gi in GATE_ORDER]
                for ut, gi in order:
                    nc.tensor.matmul(
                        ps_all[ut][gi][:],
                        wx_ap(kt, gi, ut),
                        x0c[kt],
                        start=(kt == 0),
                        stop=False,
                    )
            for kt in range(KH):
                for ut in range(UT):
                    for gi in GATE_ORDER:
                        c0 = gi * US + ut * 128
                        nc.tensor.matmul(
                            ps_all[ut][gi][:],
                            wh_t[kt][:, c0 : c0 + 128],
                            h0c[kt],
                            start=False,
                            stop=(kt == KH - 1),
                        )
            # --- n = 1 loads issued before n0's elementwise DMAs so the
            # first n1 chunk doesn't queue behind them on the ring (kills the
            # n0->n1 transition bubble). x from per-chunk tiles (reusing the
            # n=0 chunk buffers, whose n0 readers are all issued above), h
            # from a whole tile.
            x1c = {}
            for (k0, k1) in chunks:
                xc = apool.tile(
                    [128, k1 - k0, NB], BF16, tag=f"x0c{k0}", bufs=1, name=f"x1c{k0}"
                )
                nc.sync.dma_start(out=xc[:], in_=xT_r[:, k0:k1, bass.ts(1, NB)])
                for kt in range(k0, k1):
                    x1c[kt] = xc[:, kt - k0, :]
            h1_sb = apool.tile([128, KH, NB], BF16, tag="h_sb", name="h1_sb")
            nc.sync.dma_start(out=h1_sb[:], in_=hT_r[:, :, bass.ts(1, NB)])

            for ut in range(UT):
                elementwise(ps_all[ut], 0, ut)
            for ut in range(UT):
                pss = [
                    ppool.tile([128, NB], F32, tag="ps", name=f"ps{gi}")
                    for gi in range(4)
                ]
                for gi in GATE_ORDER:
                    c0 = gi * US + ut * 128
                    for kt in range(KX):
                        nc.tensor.matmul(
                            pss[gi][:],
                            wx_ap(kt, gi, ut),
                            x1c[kt],
                            start=(kt == 0),
                            stop=False,
                        )
                    for kt in range(KH):
                        nc.tensor.matmul(
                            pss[gi][:],
                            wh_t[kt][:, c0 : c0 + 128],
                            h1_sb[:, kt, :],
                            start=False,
                            stop=(kt == KH - 1),
                        )
                elementwise(pss, 1, ut)

            # --- n = 2..7 in weight-stationary pairs: per (ut, wave, k) one
            # stationary weight slice feeds both batch tiles' matmuls
            # back-to-back (walrus skips the second LDWEIGHTS). Each wave
            # runs its x phase before its h phase; x tiles die after ut=1's
            # wave-B x phase so the next pair's x prefetch fits in the same
            # two buffers.
            for (na, nb) in ((2, 3), (4, 5), (6, 7)):
                last_pair = nb == NT - 1
                xa = apool.tile([128, KX, NB], BF16, tag="x_sb", name=f"x{na}")
                nc.sync.dma_start(out=xa[:], in_=xT_r[:, :, bass.ts(na, NB)])
                xb = apool.tile([128, KX, NB], BF16, tag="x_sb", name=f"x{nb}")
                nc.sync.dma_start(out=xb[:], in_=xT_r[:, :, bass.ts(nb, NB)])
                ha = apool.tile([128, KH, NB], BF16, tag="h_sb", name=f"h{na}")
                nc.sync.dma_start(out=ha[:], in_=hT_r[:, :, bass.ts(na, NB)])
                hb = apool.tile([128, KH, NB], BF16, tag="h_sb", name=f"h{nb}")
                nc.sync.dma_start(out=hb[:], in_=hT_r[:, :, bass.ts(nb, NB)])

                for ut in range(UT):
                    if last_pair and ut == UT - 1:
                        # kernel finale: per-tile gate-outer (no weight
                        # sharing); tile b's o-gate is split 384/128 so only
                        # a 128-wide act+mul+store trails the last matmul
                        usl = slice(ut * 128, (ut + 1) * 128)
                        pss = [
                            ppool.tile([128, NB], F32, tag="ps", name=f"ps{gi}")
                            for gi in range(4)
                        ]
                        for gi in GATE_ORDER:
                            c0 = gi * US + ut * 128
                            for kt in range(KX):
                                nc.tensor.matmul(
                                    pss[gi][:], wx_ap(kt, gi, ut), xa[:, kt, :],
                                    start=(kt == 0), stop=False,
                                )
                            for kt in range(KH):
                                nc.tensor.matmul(
                                    pss[gi][:], wh_t[kt][:, c0 : c0 + 128],
                                    ha[:, kt, :],
                                    start=False, stop=(kt == KH - 1),
                                )
                        elementwise(pss, na, ut)

                        nslb = bass.ts(nb, NB)
                        psb = [
                            ppool.tile([128, NB], F32, tag="ps", name=f"psb{gi}")
                            for gi in range(4)
                        ]
                        for gi in (3, 1, 0):
                            c0 = gi * US + ut * 128
                            for kt in range(KX):
                                nc.tensor.matmul(
                                    psb[gi][:], wx_ap(kt, gi, ut), xb[:, kt, :],
                                    start=(kt == 0), stop=False,
                                )
                            for kt in range(KH):
                                nc.tensor.matmul(
                                    psb[gi][:], wh_t[kt][:, c0 : c0 + 128],
                                    hb[:, kt, :],
                                    start=False, stop=(kt == KH - 1),
                                )
                        # combine chain for c_new runs during the o loops
                        c_sb = epool.tile([128, NB], BF16, tag="c_sb", name="c_sb")
                        nc.sync.dma_start(out=c_sb[:], in_=cT[usl, nslb])
                        g_t = act_gate(psb[3], 3, ut, "g_t")
                        i_t = act_gate(psb[1], 1, ut, "i_t")
                        nc.vector.tensor_mul(i_t[:], i_t[:], g_t[:])
                        f_t = act_gate(psb[0], 0, ut, "f_t")
                        nc.vector.tensor_mul(f_t[:], f_t[:], c_sb[:])
                        cn = epool.tile([128, NB], BF16, tag="cn", name="cn")
                        nc.vector.tensor_add(cn[:], f_t[:], i_t[:])
                        nc.sync.dma_start(out=c_newT[usl, nslb], in_=cn[:])
                        tn = epool.tile([128, NB], BF16, tag="tnb", name="tn")
                        nc.scalar.activation(tn[:], cn[:], AF.Tanh)
                        # o gate, wide part: evacuates while the narrow part
                        # is still matmuling
                        ps_o2 = ppool.tile([128, NB], F32, tag="ps", name="ps_o2")
                        c0o = 2 * US + ut * 128
                        for kt in range(KX):
                            nc.tensor.matmul(
                                psb[2][:, 0:384], wx_ap(kt, 2, ut),
                                xb[:, kt, 0:384],
                                start=(kt == 0), stop=False,
                            )
                        for kt in range(KH):
                            nc.tensor.matmul(
                                psb[2][:, 0:384], wh_t[kt][:, c0o : c0o + 128],
                                hb[:, kt, 0:384],
                                start=False, stop=(kt == KH - 1),
                            )
                        o1 = act_gate(psb[2][:, 0:384], 2, ut, "o1", w=384)
                        nc.vector.tensor_mul(o1[:], o1[:], tn[:, 0:384])
                        nc.sync.dma_start(
                            out=h_newT[usl, nb * NB : nb * NB + 384], in_=o1[:]
                        )
                        for kt in range(KX):
                            nc.tensor.matmul(
                                ps_o2[:, 0:128], wx_ap(kt, 2, ut),
                                xb[:, kt, 384:512],
                                start=(kt == 0), stop=False,
                            )
                        for kt in range(KH):
                            nc.tensor.matmul(
                                ps_o2[:, 0:128], wh_t[kt][:, c0o : c0o + 128],
                                hb[:, kt, 384:512],
                                start=False, stop=(kt == KH - 1),
                            )
                        o2 = act_gate(ps_o2[:, 0:128], 2, ut, "o2", w=128)
                        nc.vector.tensor_mul(o2[:], o2[:], tn[:, 384:512])
                        nc.sync.dma_start(
                            out=h_newT[usl, nb * NB + 384 : (nb + 1) * NB],
                            in_=o2[:],
                        )
                        continue
                    psA = {
                        gi: [
                            ppool.tile([128, NB], F32, tag="ps", name=f"ps{gi}{t}")
                            for t in "ab"
                        ]
                        for gi in WAVE_A
                    }
                    for kt in range(KX):
                        for gi in WAVE_A:
                            w = wx_ap(kt, gi, ut)
                            nc.tensor.matmul(
                                psA[gi][0][:], w, xa[:, kt, :],
                                start=(kt == 0), stop=False,
                            )
                            nc.tensor.matmul(
                                psA[gi][1][:], w, xb[:, kt, :],
                                start=(kt == 0), stop=False,
                            )
                    psB = {
                        gi: [
                            ppool.tile([128, NB], F32, tag="ps", name=f"ps{gi}{t}")
                            for t in "ab"
                        ]
                        for gi in WAVE_B
                    }
                    for kt in range(KX):
                        for gi in WAVE_B:
                            w = wx_ap(kt, gi, ut)
                            nc.tensor.matmul(
                                psB[gi][0][:], w, xa[:, kt, :],
                                start=(kt == 0), stop=False,
                            )
                            nc.tensor.matmul(
                                psB[gi][1][:], w, xb[:, kt, :],
                                start=(kt == 0), stop=False,
                            )
                    for kt in range(KH):
                        for gi in WAVE_A:
                            c0 = gi * US + ut * 128
                            w = wh_t[kt][:, c0 : c0 + 128]
                            nc.tensor.matmul(
                                psA[gi][0][:], w, ha[:, kt, :],
                                start=False, stop=(kt == KH - 1),
                            )
                            nc.tensor.matmul(
                                psA[gi][1][:], w, hb[:, kt, :],
                                start=False, stop=(kt == KH - 1),
                            )
                    ig_a = stage1(psA, 0, ut, f"a{ut}")
                    ig_b = stage1(psA, 1, ut, f"b{ut}")
                    for kt in range(KH):
                        for gi in WAVE_B:
                            c0 = gi * US + ut * 128
                            w = wh_t[kt][:, c0 : c0 + 128]
                            nc.tensor.matmul(
                                psB[gi][0][:], w, ha[:, kt, :],
                                start=False, stop=(kt == KH - 1),
                            )
                            nc.tensor.matmul(
                                psB[gi][1][:], w, hb[:, kt, :],
                                start=False, stop=(kt == KH - 1),
                            )
                    stage2(psB, 0, ig_a, na, ut)
                    stage2(psB, 1, ig_b, nb, ut)
    _split_excess_waits(nc)
    return nc


_NC_CACHE = None


def _get_nc():
    global _NC_CACHE
    if _NC_CACHE is None:
        _NC_CACHE = build_nc()
    return _NC_CACHE


def make_in_maps(x, h, c, Wxf, Wxi, Wxo, Wxg, bf, bi, bo, bg, Whf, Whi, Who, Whg):
    bf16 = ml_dtypes.bfloat16
    xT = np.ascontiguousarray(np.asarray(x, np.float32).T).astype(bf16)
    hT = np.ascontiguousarray(np.asarray(h, np.float32).T).astype(bf16)
    c = np.asarray(c, np.float32)
    Wx = np.stack([np.asarray(w, np.float32) for w in (Wxf, Wxi, Wxo, Wxg)])
    Wh = np.stack([np.asarray(w, np.float32) for w in (Whf, Whi, Who, Whg)])
    bias = np.stack([np.asarray(v, np.float32) for v in (bf, bi, bo, bg)])

    in_maps = []
    for i in range(N_CORES):
        s = slice(i * US, (i + 1) * US)
        wx_i = np.concatenate([Wx[g, :, s] for g in range(4)], axis=1).astype(bf16)
        wh_i = np.concatenate([Wh[g, :, s] for g in range(4)], axis=1).astype(bf16)
        b_i = np.concatenate([bias[g, s] for g in range(4)])  # [1024]
        b_i = np.ascontiguousarray(b_i.reshape(4 * UT, 128).T)  # [128, 8]
        cT_i = np.ascontiguousarray(c[:, s].T).astype(bf16)  # [US, B]
        in_maps.append(
            {"xT": xT, "hT": hT, "wx": wx_i, "wh": wh_i, "bias": b_i, "cT": cT_i}
        )
    return in_maps


def run(in_maps, **kwargs):
    nc = _get_nc()
    return run_bass_kernel_spmd(nc, in_maps, list(range(N_CORES)), **kwargs)


def gather(results):
    h_new = np.empty((B, U), np.float32)
    c_new = np.empty((B, U), np.float32)
    for i in range(N_CORES):
        s = slice(i * US, (i + 1) * US)
        h_new[:, s] = results[i]["h_newT"].astype(np.float32).T
        c_new[:, s] = results[i]["c_newT"].astype(np.float32).T
    return h_new, c_new


def kernel(**inputs):
    res = run(make_in_maps(**inputs))
    return gather(res.results)
